# revision 3
# baseline (speedup 1.0000x reference)
"""Single-head causal attention kernel for Trainium2, 8-core data parallel.

Problem: x [8, 2048, 1024], Wk/Wq/Wv [64, 1024] ->
  out[b] = softmax(causal((x[b] @ Wq.T) @ (x[b] @ Wk.T).T / 8)) @ (x[b] @ Wv.T)

Sharding: one batch element per NeuronCore (data parallel across batch).

Per-core dataflow (all SBUF-resident, fp32):
  - host supplies xT = x[b].T [1024, 2048] so the embedding (contraction) dim
    lands on SBUF partitions directly; weights supplied pre-transposed and
    q/k fused: wqk = [Wq.T | Wk.T] [1024, 128].
  - qT/kT [64, 2048] computed with ONE packed matmul chain (stationary
    [128e, 128(q|k)] at full PE width); vT [64, 2048] separately.
  - v is re-transposed to natural [t_k, 64] via PE transpose, with a column
    of ones appended -> ve [t_k, 65]; the ones column makes the attention
    output matmul produce softmax row-sums for free.
  - scores are computed TRANSPOSED, sT[t_k, t_q] = k_j @ qT, so that
    P = exp(sT/8) needs no per-column bias (scores are bounded ~[-3, 4],
    max-subtraction is unnecessary in fp32) and P feeds the output matmul
    as the moving operand with no further transposes:
      out_psum[65, t_q] += ve_j.T @ P_j   (accumulated over key blocks j)
  - causal structure at 128-block granularity: only blocks t_k <= t_q are
    computed (136 of 256); the diagonal block is masked with a 0/1
    upper-triangular mask after exp.
  - device output is the unnormalized [65, 2048] (64 head dims + sums row);
    host divides by the sums row and transposes (0.26% of the FLOPs).
"""
import sys

for _p in ("/opt/trn_rl_repo",):
    if _p not in sys.path:
        sys.path.insert(0, _p)

import numpy as np
from contextlib import ExitStack

import concourse.bass as bass
import concourse.tile as tile
from concourse import bacc, mybir
from concourse.bass_utils import run_bass_kernel_spmd

FP = mybir.dt.float32
B, T, E, H = 8, 2048, 1024, 64
NE = E // 128          # 8 e-tiles (contraction)
NT = T // 128          # 16 token tiles
CH = 512               # qkv column chunk (= one PSUM bank of fp32)
NC_CHUNKS = T // CH    # 4
SCALE = 1.0 / np.sqrt(H)  # 0.125

_CACHE = {}


def _build_nc():
    nc = bacc.Bacc(None, target_bir_lowering=False, debug=False)

    xt_d = nc.dram_tensor("xt", [E, T], FP, kind="ExternalInput")
    wqk_d = nc.dram_tensor("wqk", [E, 2 * H], FP, kind="ExternalInput")
    wv_d = nc.dram_tensor("wv", [E, H], FP, kind="ExternalInput")
    mask_d = nc.dram_tensor("mask", [128, 128], FP, kind="ExternalInput")
    id_d = nc.dram_tensor("ident", [64, 64], FP, kind="ExternalInput")
    out_d = nc.dram_tensor("out", [H + 1, T], FP, kind="ExternalOutput")

    with tile.TileContext(nc) as tc, ExitStack() as ctx:
        const = ctx.enter_context(tc.tile_pool(name="const", bufs=1))
        ve_pool = ctx.enter_context(tc.tile_pool(name="ve", bufs=NT))
        p_pool = ctx.enter_context(tc.tile_pool(name="pstripe", bufs=2))
        qk_psum = ctx.enter_context(
            tc.tile_pool(name="qk_ps", bufs=1, space=bass.MemorySpace.PSUM))
        vt_psum = ctx.enter_context(
            tc.tile_pool(name="vt_ps", bufs=1, space=bass.MemorySpace.PSUM))
        work_psum = ctx.enter_context(
            tc.tile_pool(name="work_ps", bufs=2, space=bass.MemorySpace.PSUM))
        out_psum = ctx.enter_context(
            tc.tile_pool(name="out_ps", bufs=1, space=bass.MemorySpace.PSUM))

        # ---- SBUF tensors ----
        xt_sb = const.tile([128, NE * T], FP)       # xT, e-tile-major
        wqk_sb = const.tile([128, NE * 2 * H], FP)
        wv_sb = const.tile([128, NE * H], FP)
        mask_sb = const.tile([128, 128], FP)
        id_sb = const.tile([64, 64], FP)
        qk_sb = const.tile([128, T], FP)            # rows 0:64 qT, 64:128 kT
        k_lo = const.tile([64, T], FP)              # kT moved to partitions 0:64
        vT_sb = const.tile([64, T], FP)
        out_sb = const.tile([H + 1, T], FP)
        ve = [ve_pool.tile([128, H + 1], FP, tag="ve", name=f"ve{t}")
              for t in range(NT)]

        # ---- input DMAs ----
        nc.sync.dma_start(
            wqk_sb[:].rearrange("p (ne m) -> p ne m", m=2 * H),
            wqk_d.ap().rearrange("(ne p) m -> p ne m", p=128))
        nc.sync.dma_start(
            wv_sb[:].rearrange("p (ne m) -> p ne m", m=H),
            wv_d.ap().rearrange("(ne p) m -> p ne m", p=128))
        nc.sync.dma_start(mask_sb[:], mask_d.ap())
        nc.sync.dma_start(id_sb[:], id_d.ap())
        # x chunks, last column chunk first (attention unlocks high-j first)
        xt_in = xt_d.ap().rearrange("(ne p) t -> p ne t", p=128)
        xt_out = xt_sb[:].rearrange("p (ne t) -> p ne t", t=T)
        for n in range(NC_CHUNKS - 1, -1, -1):
            nc.sync.dma_start(
                xt_out[:, :, n * CH:(n + 1) * CH],
                xt_in[:, :, n * CH:(n + 1) * CH])

        for t in range(NT):
            nc.gpsimd.memset(ve[t][:, H:H + 1], 1.0)

        out_ps = out_psum.tile([H + 1, T], FP)

        # 512-aligned piece list for the [j*128, T) column stripe of block j:
        # a leading 128-wide diagonal piece, then pieces up to the next
        # 512 boundary, then full 512s (PSUM-bank-aligned for out_ps).
        def stripe_pieces(j):
            pieces = [(j * 128, (j + 1) * 128)]
            c = (j + 1) * 128
            while c < T:
                e = min((c // CH + 1) * CH, T)
                pieces.append((c, e))
                c = e
            return pieces

        for n in range(NC_CHUNKS - 1, -1, -1):
            cols = bass.ts(n, CH)
            # -- packed q|k projection for this column chunk --
            qk_ps = qk_psum.tile([128, CH], FP, tag="qk")
            for e in range(NE):
                nc.tensor.matmul(
                    qk_ps[:],
                    wqk_sb[:, bass.ts(e, 2 * H)],
                    xt_sb[:, e * T + n * CH: e * T + (n + 1) * CH],
                    start=(e == 0), stop=(e == NE - 1))
            nc.scalar.copy(qk_sb[:, cols], qk_ps[:])
            # move kT rows down to partitions 0:64 (partition remap via DMA)
            nc.scalar.dma_start(k_lo[:, cols], qk_sb[64:128, cols])
            # -- vT projection --
            vt_ps = vt_psum.tile([64, CH], FP, tag="vt")
            for e in range(NE):
                nc.tensor.matmul(
                    vt_ps[:],
                    wv_sb[:, bass.ts(e, H)],
                    xt_sb[:, e * T + n * CH: e * T + (n + 1) * CH],
                    start=(e == 0), stop=(e == NE - 1))
            nc.scalar.copy(vT_sb[:, cols], vt_ps[:])
            # -- v natural tiles (PE transpose) + ones column --
            for t in range(4 * n, 4 * n + 4):
                tr_ps = work_psum.tile([128, CH], FP, tag="work")
                nc.tensor.transpose(
                    tr_ps[:, 0:H], vT_sb[:, bass.ts(t, 128)], id_sb[:])
                nc.vector.tensor_copy(ve[t][:, 0:H], tr_ps[:, 0:H])

            # -- attention for key blocks j in this chunk (descending) --
            for j in range(4 * n + 3, 4 * n - 1, -1):
                pieces = stripe_pieces(j)
                stripe = p_pool.tile([128, T], FP, tag="p")
                for (c0, c1) in pieces:
                    s_ps = work_psum.tile([128, CH], FP, tag="work")
                    nc.tensor.matmul(
                        s_ps[:, 0:c1 - c0],
                        k_lo[:, bass.ts(j, 128)],
                        qk_sb[0:64, c0:c1],
                        start=True, stop=True)
                    nc.scalar.activation(
                        stripe[:, c0 - j * 128: c1 - j * 128],
                        s_ps[:, 0:c1 - c0],
                        mybir.ActivationFunctionType.Exp,
                        scale=float(SCALE))
                # mask the diagonal block (upper-tri keep in [t_k, t_q])
                nc.vector.tensor_mul(
                    stripe[:, 0:128], stripe[:, 0:128], mask_sb[:])
                # out_psum[:, c0:c1] += ve_j.T @ P_j
                # start=True zeroes the target PSUM *bank*, so only the first
                # stripe to touch a bank (j % 4 == 3, descending) may set it;
                # stop marks the last write per bank (all j==0 pieces except
                # the diagonal one, whose bank is finished by the next piece).
                for (c0, c1) in pieces:
                    nc.tensor.matmul(
                        out_ps[:, c0:c1],
                        ve[j][:],
                        stripe[:, c0 - j * 128: c1 - j * 128],
                        start=(c0 == j * 128 and j % 4 == 3),
                        stop=(j == 0 and c0 != 0),
                        skip_group_check=True)

        for n in range(NC_CHUNKS):
            nc.vector.tensor_copy(out_sb[:, bass.ts(n, CH)],
                                  out_ps[:, bass.ts(n, CH)])
        nc.sync.dma_start(out_d.ap(), out_sb[:])

    nc.compile()
    return nc


def _get_nc():
    if "nc" not in _CACHE:
        _CACHE["nc"] = _build_nc()
    return _CACHE["nc"]


def kernel(x, Wk, Wq, Wv):
    x = np.ascontiguousarray(x, dtype=np.float32)
    assert x.shape == (B, T, E)
    nc = _get_nc()

    wqk = np.ascontiguousarray(
        np.concatenate([Wq.T, Wk.T], axis=1), dtype=np.float32)   # [E, 128]
    wv = np.ascontiguousarray(Wv.T, dtype=np.float32)             # [E, 64]
    mask = np.triu(np.ones((128, 128), dtype=np.float32))         # keep t_k <= t_q
    ident = np.eye(64, dtype=np.float32)

    in_maps = []
    for b in range(B):
        in_maps.append({
            "xt": np.ascontiguousarray(x[b].T),
            "wqk": wqk,
            "wv": wv,
            "mask": mask,
            "ident": ident,
        })

    res = run_bass_kernel_spmd(nc, in_maps, list(range(B)))
    out = np.empty((B, T, H), dtype=np.float32)
    for b in range(B):
        y = res.results[b]["out"]          # [65, T] unnormalized
        out[b] = (y[:H] / y[H:H + 1]).T
    return out


def run_traced(x, Wk, Wq, Wv):
    """Like kernel() but with NTFF profiling; returns (out, BassKernelResults)."""
    import types
    import antenv
    if "antenv.axon_hooks" not in sys.modules:
        hooks_mod = types.ModuleType("antenv.axon_hooks")
        _HOOK = [None]
        hooks_mod.set_axon_ntff_profile_hook = lambda h: _HOOK.__setitem__(0, h)
        hooks_mod.get_axon_ntff_profile_hook = lambda: _HOOK[0]
        sys.modules["antenv.axon_hooks"] = hooks_mod
        antenv.axon_hooks = hooks_mod
        from trn_agent_boot.trn_boot import _ntff_profile_via_ctypes
        hooks_mod.set_axon_ntff_profile_hook(
            _ntff_profile_via_ctypes("/opt/axon/libaxon_pjrt.so"))

    x = np.ascontiguousarray(x, dtype=np.float32)
    nc = _get_nc()
    wqk = np.ascontiguousarray(
        np.concatenate([Wq.T, Wk.T], axis=1), dtype=np.float32)
    wv = np.ascontiguousarray(Wv.T, dtype=np.float32)
    mask = np.triu(np.ones((128, 128), dtype=np.float32))
    ident = np.eye(64, dtype=np.float32)
    in_maps = [{
        "xt": np.ascontiguousarray(x[b].T),
        "wqk": wqk, "wv": wv, "mask": mask, "ident": ident,
    } for b in range(B)]
    res = run_bass_kernel_spmd(
        nc, in_maps, list(range(B)), trace=True, trace_cores=[0])
    out = np.empty((B, T, H), dtype=np.float32)
    for b in range(B):
        y = res.results[b]["out"]
        out[b] = (y[:H] / y[H:H + 1]).T
    return out, res


# revision 6
# speedup vs baseline: 1.4808x; 1.4808x over previous
"""Single-head causal attention kernel for Trainium2, 8-core data parallel.

Problem: x [8, 2048, 1024], Wk/Wq/Wv [64, 1024] ->
  out[b] = softmax(causal((x[b] @ Wq.T) @ (x[b] @ Wk.T).T / 8)) @ (x[b] @ Wv.T)

Sharding: one batch element per NeuronCore (data parallel across batch).

Per-core dataflow (all SBUF-resident, fp32):
  - host supplies xT = x[b].T [1024, 2048] so the embedding (contraction) dim
    lands on SBUF partitions directly; weights supplied pre-transposed and
    q/k fused: wqk = [Wq.T | Wk.T] [1024, 128].
  - qT/kT [64, 2048] computed with ONE packed matmul chain (stationary
    [128e, 128(q|k)] at full PE width); vT [64, 2048] separately.
  - v is re-transposed to natural [t_k, 64] via PE transpose, with a column
    of ones appended -> ve [t_k, 65]; the ones column makes the attention
    output matmul produce softmax row-sums for free.
  - scores are computed TRANSPOSED, sT[t_k, t_q] = k_j @ qT, so that
    P = exp(sT/8) needs no per-column bias (scores are bounded ~[-3, 4],
    max-subtraction is unnecessary in fp32) and P feeds the output matmul
    as the moving operand with no further transposes:
      out_psum[65, t_q] += ve_j.T @ P_j   (accumulated over key blocks j)
  - causal structure at 128-block granularity: only blocks t_k <= t_q are
    computed (136 of 256); the diagonal block is masked with a 0/1
    upper-triangular mask after exp.
  - device output is the unnormalized [65, 2048] (64 head dims + sums row);
    host divides by the sums row and transposes (0.26% of the FLOPs).
"""
import sys

for _p in ("/opt/trn_rl_repo",):
    if _p not in sys.path:
        sys.path.insert(0, _p)

import numpy as np
from contextlib import ExitStack

import concourse.bass as bass
import concourse.tile as tile
from concourse import bacc, mybir
from concourse.bass_utils import run_bass_kernel_spmd

FP = mybir.dt.float32
FPR = mybir.dt.float32r
B, T, E, H = 8, 2048, 1024, 64
NE = E // 128          # 8 e-tiles (contraction)
NT = T // 128          # 16 token tiles
CH = 512               # qkv column chunk (= one PSUM bank of fp32)
NC_CHUNKS = T // CH    # 4
SCALE = 1.0 / np.sqrt(H)  # 0.125

_CACHE = {}


def _to_fp32r(a):
    """Round fp32 to the fp32r grid (11 mantissa bits, round-to-nearest)."""
    u = np.ascontiguousarray(a, dtype=np.float32).view(np.uint32)
    u = (u + (((u >> 12) & 1) + 0x7FF)) & np.uint32(0xFFFFF000)
    return u.view(np.float32)


def _build_nc():
    nc = bacc.Bacc(None, target_bir_lowering=False, debug=False)

    xt_d = nc.dram_tensor("xt", [E, T], FPR, kind="ExternalInput")
    wqk_d = nc.dram_tensor("wqk", [E, 2 * H], FPR, kind="ExternalInput")
    wv_d = nc.dram_tensor("wv", [E, H], FPR, kind="ExternalInput")
    mask_d = nc.dram_tensor("mask", [128, 128], FPR, kind="ExternalInput")
    id_d = nc.dram_tensor("ident", [64, 64], FPR, kind="ExternalInput")
    ones_d = nc.dram_tensor("ones", [128, 1], FPR, kind="ExternalInput")
    out_d = nc.dram_tensor("out", [H + 1, T], FP, kind="ExternalOutput")

    with tile.TileContext(nc) as tc, ExitStack() as ctx:
        const = ctx.enter_context(tc.tile_pool(name="const", bufs=1))
        ve_pool = ctx.enter_context(tc.tile_pool(name="ve", bufs=NT))
        p_pool = ctx.enter_context(tc.tile_pool(name="pstripe", bufs=2))
        qk_psum = ctx.enter_context(
            tc.tile_pool(name="qk_ps", bufs=1, space=bass.MemorySpace.PSUM))
        vt_psum = ctx.enter_context(
            tc.tile_pool(name="vt_ps", bufs=1, space=bass.MemorySpace.PSUM))
        work_psum = ctx.enter_context(
            tc.tile_pool(name="work_ps", bufs=2, space=bass.MemorySpace.PSUM))
        out_psum = ctx.enter_context(
            tc.tile_pool(name="out_ps", bufs=1, space=bass.MemorySpace.PSUM))

        # ---- SBUF tensors ----
        xt_sb = const.tile([128, NE * T], FPR)       # xT, e-tile-major
        wqk_sb = const.tile([128, NE * 2 * H], FPR)
        wv_sb = const.tile([128, NE * H], FPR)
        mask_sb = const.tile([128, 128], FPR)
        id_sb = const.tile([64, 64], FPR)
        qk_sb = const.tile([128, T], FPR)            # rows 0:64 qT, 64:128 kT
        k_lo = const.tile([64, T], FPR)              # kT moved to partitions 0:64
        vT_sb = const.tile([64, T], FPR)
        out_sb = const.tile([H + 1, T], FP)
        ve = [ve_pool.tile([128, H + 1], FPR, tag="ve", name=f"ve{t}")
              for t in range(NT)]

        # ---- input DMAs ----
        nc.sync.dma_start(
            wqk_sb[:].rearrange("p (ne m) -> p ne m", m=2 * H),
            wqk_d.ap().rearrange("(ne p) m -> p ne m", p=128))
        nc.sync.dma_start(
            wv_sb[:].rearrange("p (ne m) -> p ne m", m=H),
            wv_d.ap().rearrange("(ne p) m -> p ne m", p=128))
        nc.sync.dma_start(mask_sb[:], mask_d.ap())
        nc.sync.dma_start(id_sb[:], id_d.ap())
        # x chunks, last column chunk first (attention unlocks high-j first)
        xt_in = xt_d.ap().rearrange("(ne p) t -> p ne t", p=128)
        xt_out = xt_sb[:].rearrange("p (ne t) -> p ne t", t=T)
        for n in range(NC_CHUNKS - 1, -1, -1):
            nc.sync.dma_start(
                xt_out[:, :, n * CH:(n + 1) * CH],
                xt_in[:, :, n * CH:(n + 1) * CH])

        for t in range(NT):
            nc.scalar.dma_start(ve[t][:, H:H + 1], ones_d.ap())

        out_ps = out_psum.tile([H + 1, T], FP)

        # 512-aligned piece list for the [j*128, T) column stripe of block j:
        # a leading 128-wide diagonal piece, then pieces up to the next
        # 512 boundary, then full 512s (PSUM-bank-aligned for out_ps).
        def stripe_pieces(j):
            pieces = [(j * 128, (j + 1) * 128)]
            c = (j + 1) * 128
            while c < T:
                e = min((c // CH + 1) * CH, T)
                pieces.append((c, e))
                c = e
            return pieces

        for n in range(NC_CHUNKS - 1, -1, -1):
            cols = bass.ts(n, CH)
            # -- packed q|k projection for this column chunk --
            qk_ps = qk_psum.tile([128, CH], FP, tag="qk")
            for e in range(NE):
                nc.tensor.matmul(
                    qk_ps[:],
                    wqk_sb[:, bass.ts(e, 2 * H)],
                    xt_sb[:, e * T + n * CH: e * T + (n + 1) * CH],
                    start=(e == 0), stop=(e == NE - 1))
            nc.scalar.copy(qk_sb[:, cols], qk_ps[:])
            # move kT rows down to partitions 0:64 (partition remap via DMA)
            nc.scalar.dma_start(k_lo[:, cols], qk_sb[64:128, cols])
            # -- vT projection --
            vt_ps = vt_psum.tile([64, CH], FP, tag="vt")
            for e in range(NE):
                nc.tensor.matmul(
                    vt_ps[:],
                    wv_sb[:, bass.ts(e, H)],
                    xt_sb[:, e * T + n * CH: e * T + (n + 1) * CH],
                    start=(e == 0), stop=(e == NE - 1))
            nc.scalar.copy(vT_sb[:, cols], vt_ps[:])
            # -- v natural tiles (PE transpose) + ones column --
            for t in range(4 * n, 4 * n + 4):
                tr_ps = work_psum.tile([128, CH], FPR, tag="work", name="tr_ps")
                nc.tensor.transpose(
                    tr_ps[:, 0:H], vT_sb[:, bass.ts(t, 128)], id_sb[:])
                nc.vector.tensor_copy(ve[t][:, 0:H], tr_ps[:, 0:H])

            # -- attention for key blocks j in this chunk (descending) --
            for j in range(4 * n + 3, 4 * n - 1, -1):
                pieces = stripe_pieces(j)
                stripe = p_pool.tile([128, T], FPR, tag="p")
                for (c0, c1) in pieces:
                    s_ps = work_psum.tile([128, CH], FP, tag="work")
                    nc.tensor.matmul(
                        s_ps[:, 0:c1 - c0],
                        k_lo[:, bass.ts(j, 128)],
                        qk_sb[0:64, c0:c1],
                        start=True, stop=True)
                    nc.scalar.activation(
                        stripe[:, c0 - j * 128: c1 - j * 128],
                        s_ps[:, 0:c1 - c0],
                        mybir.ActivationFunctionType.Exp,
                        scale=float(SCALE))
                # mask the diagonal block (upper-tri keep in [t_k, t_q])
                nc.vector.tensor_mul(
                    stripe[:, 0:128], stripe[:, 0:128], mask_sb[:])
                # out_psum[:, c0:c1] += ve_j.T @ P_j
                # start=True zeroes the target PSUM *bank*, so only the first
                # stripe to touch a bank (j % 4 == 3, descending) may set it;
                # stop marks the last write per bank (all j==0 pieces except
                # the diagonal one, whose bank is finished by the next piece).
                for (c0, c1) in pieces:
                    nc.tensor.matmul(
                        out_ps[:, c0:c1],
                        ve[j][:],
                        stripe[:, c0 - j * 128: c1 - j * 128],
                        start=(c0 == j * 128 and j % 4 == 3),
                        stop=(j == 0 and c0 != 0),
                        skip_group_check=True)

        for n in range(NC_CHUNKS):
            nc.vector.tensor_copy(out_sb[:, bass.ts(n, CH)],
                                  out_ps[:, bass.ts(n, CH)])
        nc.sync.dma_start(out_d.ap(), out_sb[:])

    nc.compile()
    return nc


def _get_nc():
    if "nc" not in _CACHE:
        _CACHE["nc"] = _build_nc()
    return _CACHE["nc"]


def kernel(x, Wk, Wq, Wv):
    x = np.ascontiguousarray(x, dtype=np.float32)
    assert x.shape == (B, T, E)
    nc = _get_nc()

    wqk = _to_fp32r(np.concatenate([Wq.T, Wk.T], axis=1))        # [E, 128]
    wv = _to_fp32r(Wv.T)                                          # [E, 64]
    mask = np.triu(np.ones((128, 128), dtype=np.float32))         # keep t_k <= t_q
    ident = np.eye(64, dtype=np.float32)
    ones = np.ones((128, 1), dtype=np.float32)

    in_maps = []
    for b in range(B):
        in_maps.append({
            "xt": _to_fp32r(x[b].T),
            "wqk": wqk,
            "wv": wv,
            "mask": mask,
            "ident": ident,
            "ones": ones,
        })

    res = run_bass_kernel_spmd(nc, in_maps, list(range(B)))
    out = np.empty((B, T, H), dtype=np.float32)
    for b in range(B):
        y = res.results[b]["out"]          # [65, T] unnormalized
        out[b] = (y[:H] / y[H:H + 1]).T
    return out


def run_traced(x, Wk, Wq, Wv):
    """Like kernel() but with NTFF profiling; returns (out, BassKernelResults)."""
    import types
    import antenv
    if "antenv.axon_hooks" not in sys.modules:
        hooks_mod = types.ModuleType("antenv.axon_hooks")
        _HOOK = [None]
        hooks_mod.set_axon_ntff_profile_hook = lambda h: _HOOK.__setitem__(0, h)
        hooks_mod.get_axon_ntff_profile_hook = lambda: _HOOK[0]
        sys.modules["antenv.axon_hooks"] = hooks_mod
        antenv.axon_hooks = hooks_mod
        from trn_agent_boot.trn_boot import _ntff_profile_via_ctypes
        hooks_mod.set_axon_ntff_profile_hook(
            _ntff_profile_via_ctypes("/opt/axon/libaxon_pjrt.so"))

    x = np.ascontiguousarray(x, dtype=np.float32)
    nc = _get_nc()
    wqk = _to_fp32r(np.concatenate([Wq.T, Wk.T], axis=1))
    wv = _to_fp32r(Wv.T)
    mask = np.triu(np.ones((128, 128), dtype=np.float32))
    ident = np.eye(64, dtype=np.float32)
    ones = np.ones((128, 1), dtype=np.float32)
    in_maps = [{
        "xt": _to_fp32r(x[b].T),
        "wqk": wqk, "wv": wv, "mask": mask, "ident": ident, "ones": ones,
    } for b in range(B)]
    res = run_bass_kernel_spmd(
        nc, in_maps, list(range(B)), trace=True, trace_cores=[0])
    out = np.empty((B, T, H), dtype=np.float32)
    for b in range(B):
        y = res.results[b]["out"]
        out[b] = (y[:H] / y[H:H + 1]).T
    return out, res


# revision 8
# speedup vs baseline: 1.5003x; 1.0132x over previous
"""Single-head causal attention kernel for Trainium2, 8-core data parallel.

Problem: x [8, 2048, 1024], Wk/Wq/Wv [64, 1024] ->
  out[b] = softmax(causal((x[b] @ Wq.T) @ (x[b] @ Wk.T).T / 8)) @ (x[b] @ Wv.T)

Sharding: one batch element per NeuronCore (data parallel across batch).

Per-core dataflow (all SBUF-resident, fp32):
  - host supplies xT = x[b].T [1024, 2048] so the embedding (contraction) dim
    lands on SBUF partitions directly; weights supplied pre-transposed and
    q/k fused: wqk = [Wq.T | Wk.T] [1024, 128].
  - qT/kT [64, 2048] computed with ONE packed matmul chain (stationary
    [128e, 128(q|k)] at full PE width); vT [64, 2048] separately.
  - v is re-transposed to natural [t_k, 64] via PE transpose, with a column
    of ones appended -> ve [t_k, 65]; the ones column makes the attention
    output matmul produce softmax row-sums for free.
  - scores are computed TRANSPOSED, sT[t_k, t_q] = k_j @ qT, so that
    P = exp(sT/8) needs no per-column bias (scores are bounded ~[-3, 4],
    max-subtraction is unnecessary in fp32) and P feeds the output matmul
    as the moving operand with no further transposes:
      out_psum[65, t_q] += ve_j.T @ P_j   (accumulated over key blocks j)
  - causal structure at 128-block granularity: only blocks t_k <= t_q are
    computed (136 of 256); the diagonal block is masked with a 0/1
    upper-triangular mask after exp.
  - device output is the unnormalized [65, 2048] (64 head dims + sums row);
    host divides by the sums row and transposes (0.26% of the FLOPs).
"""
import sys

for _p in ("/opt/trn_rl_repo",):
    if _p not in sys.path:
        sys.path.insert(0, _p)

import numpy as np
from contextlib import ExitStack

import concourse.bass as bass
import concourse.tile as tile
from concourse import bacc, mybir
from concourse.bass_utils import run_bass_kernel_spmd

FP = mybir.dt.float32
FPR = mybir.dt.float32r
B, T, E, H = 8, 2048, 1024, 64
NE = E // 128          # 8 e-tiles (contraction)
NT = T // 128          # 16 token tiles
CH = 512               # qkv column chunk (= one PSUM bank of fp32)
NC_CHUNKS = T // CH    # 4
SCALE = 1.0 / np.sqrt(H)  # 0.125

_CACHE = {}


def _to_fp32r(a):
    """Round fp32 to the fp32r grid (11 mantissa bits, round-to-nearest)."""
    u = np.ascontiguousarray(a, dtype=np.float32).view(np.uint32)
    u = (u + (((u >> 12) & 1) + 0x7FF)) & np.uint32(0xFFFFF000)
    return u.view(np.float32)


def _build_nc():
    nc = bacc.Bacc(None, target_bir_lowering=False, debug=False)

    xt_d = nc.dram_tensor("xt", [E, T], FPR, kind="ExternalInput")
    wqk_d = nc.dram_tensor("wqk", [E, 2 * H], FPR, kind="ExternalInput")
    wv_d = nc.dram_tensor("wv", [E, H], FPR, kind="ExternalInput")
    mask_d = nc.dram_tensor("mask", [128, 128], FPR, kind="ExternalInput")
    id_d = nc.dram_tensor("ident", [64, 64], FPR, kind="ExternalInput")
    ones_d = nc.dram_tensor("ones", [128, 1], FPR, kind="ExternalInput")
    out_d = nc.dram_tensor("out", [H + 1, T], FP, kind="ExternalOutput")

    with tile.TileContext(nc) as tc, ExitStack() as ctx:
        const = ctx.enter_context(tc.tile_pool(name="const", bufs=1))
        ve_pool = ctx.enter_context(tc.tile_pool(name="ve", bufs=NT))
        p_pool = ctx.enter_context(tc.tile_pool(name="pstripe", bufs=2))
        qk_psum = ctx.enter_context(
            tc.tile_pool(name="qk_ps", bufs=1, space=bass.MemorySpace.PSUM))
        vt_psum = ctx.enter_context(
            tc.tile_pool(name="vt_ps", bufs=1, space=bass.MemorySpace.PSUM))
        work_psum = ctx.enter_context(
            tc.tile_pool(name="work_ps", bufs=2, space=bass.MemorySpace.PSUM))
        out_psum = ctx.enter_context(
            tc.tile_pool(name="out_ps", bufs=1, space=bass.MemorySpace.PSUM))

        # ---- SBUF tensors ----
        xt_sb = const.tile([128, NE * T], FPR)       # xT, e-tile-major
        wqk_sb = const.tile([128, NE * 2 * H], FPR)
        wv_sb = const.tile([128, NE * H], FPR)
        mask_sb = const.tile([128, 128], FPR)
        id_sb = const.tile([64, 64], FPR)
        qk_sb = const.tile([128, T], FPR)            # rows 0:64 qT, 64:128 kT
        k_lo = const.tile([64, T], FPR)              # kT moved to partitions 0:64
        vT_sb = const.tile([64, T], FPR)
        out_sb = const.tile([H + 1, T], FP)
        ve = [ve_pool.tile([128, H + 1], FPR, tag="ve", name=f"ve{t}")
              for t in range(NT)]

        # ---- input DMAs ----
        nc.sync.dma_start(
            wqk_sb[:].rearrange("p (ne m) -> p ne m", m=2 * H),
            wqk_d.ap().rearrange("(ne p) m -> p ne m", p=128))
        nc.sync.dma_start(
            wv_sb[:].rearrange("p (ne m) -> p ne m", m=H),
            wv_d.ap().rearrange("(ne p) m -> p ne m", p=128))
        nc.sync.dma_start(mask_sb[:], mask_d.ap())
        nc.sync.dma_start(id_sb[:], id_d.ap())
        # x chunks, last column chunk first (attention unlocks high-j first)
        xt_in = xt_d.ap().rearrange("(ne p) t -> p ne t", p=128)
        xt_out = xt_sb[:].rearrange("p (ne t) -> p ne t", t=T)
        prev_dma = None
        for n in range(NC_CHUNKS - 1, -1, -1):
            d = nc.sync.dma_start(
                xt_out[:, :, n * CH:(n + 1) * CH],
                xt_in[:, :, n * CH:(n + 1) * CH])
            if prev_dma is not None:
                # serialize the x chunk loads: the SDMA engines otherwise
                # round-robin all queued DMAs and no chunk finishes early
                tile.add_dep_helper(d.ins, prev_dma.ins, sync=True,
                                    reason="x chunks complete in order")
            prev_dma = d

        for t in range(NT):
            nc.scalar.dma_start(ve[t][:, H:H + 1], ones_d.ap())

        out_ps = out_psum.tile([H + 1, T], FP)

        # 512-aligned piece list for the [j*128, T) column stripe of block j:
        # a leading 128-wide diagonal piece, then pieces up to the next
        # 512 boundary, then full 512s (PSUM-bank-aligned for out_ps).
        def stripe_pieces(j):
            pieces = [(j * 128, (j + 1) * 128)]
            c = (j + 1) * 128
            while c < T:
                e = min((c // CH + 1) * CH, T)
                pieces.append((c, e))
                c = e
            return pieces

        for n in range(NC_CHUNKS - 1, -1, -1):
            cols = bass.ts(n, CH)
            # -- packed q|k projection for this column chunk --
            qk_ps = qk_psum.tile([128, CH], FP, tag="qk", name="qk_ps")
            for e in range(NE):
                nc.tensor.matmul(
                    qk_ps[:],
                    wqk_sb[:, bass.ts(e, 2 * H)],
                    xt_sb[:, e * T + n * CH: e * T + (n + 1) * CH],
                    start=(e == 0), stop=(e == NE - 1))
            nc.scalar.copy(qk_sb[:, cols], qk_ps[:])
            # move kT rows down to partitions 0:64 (partition remap via DMA)
            nc.scalar.dma_start(k_lo[:, cols], qk_sb[64:128, cols])
            # -- vT projection --
            vt_ps = vt_psum.tile([64, CH], FP, tag="vt", name="vt_ps")
            for e in range(NE):
                nc.tensor.matmul(
                    vt_ps[:],
                    wv_sb[:, bass.ts(e, H)],
                    xt_sb[:, e * T + n * CH: e * T + (n + 1) * CH],
                    start=(e == 0), stop=(e == NE - 1))
            nc.scalar.copy(vT_sb[:, cols], vt_ps[:])
            # -- v natural tiles (PE transpose) + ones column --
            for t in range(4 * n, 4 * n + 4):
                tr_ps = work_psum.tile([128, CH], FPR, tag="work", name="tr_ps")
                nc.tensor.transpose(
                    tr_ps[:, 0:H], vT_sb[:, bass.ts(t, 128)], id_sb[:])
                nc.vector.tensor_copy(ve[t][:, 0:H], tr_ps[:, 0:H])

            # -- attention for key blocks j in this chunk (descending) --
            for j in range(4 * n + 3, 4 * n - 1, -1):
                pieces = stripe_pieces(j)
                stripe = p_pool.tile([128, T], FPR, tag="p")
                for (c0, c1) in pieces:
                    s_ps = work_psum.tile([128, CH], FP, tag="work")
                    nc.tensor.matmul(
                        s_ps[:, 0:c1 - c0],
                        k_lo[:, bass.ts(j, 128)],
                        qk_sb[0:64, c0:c1],
                        start=True, stop=True)
                    nc.scalar.activation(
                        stripe[:, c0 - j * 128: c1 - j * 128],
                        s_ps[:, 0:c1 - c0],
                        mybir.ActivationFunctionType.Exp,
                        scale=float(SCALE))
                # mask the diagonal block (upper-tri keep in [t_k, t_q])
                nc.vector.tensor_mul(
                    stripe[:, 0:128], stripe[:, 0:128], mask_sb[:])
                # out_psum[:, c0:c1] += ve_j.T @ P_j
                # start=True zeroes the target PSUM *bank*, so only the first
                # stripe to touch a bank (j % 4 == 3, descending) may set it;
                # stop marks the last write per bank (all j==0 pieces except
                # the diagonal one, whose bank is finished by the next piece).
                for (c0, c1) in pieces:
                    nc.tensor.matmul(
                        out_ps[:, c0:c1],
                        ve[j][:],
                        stripe[:, c0 - j * 128: c1 - j * 128],
                        start=(c0 == j * 128 and j % 4 == 3),
                        stop=(j == 0 and c0 != 0),
                        skip_group_check=True)

        for n in range(NC_CHUNKS):
            nc.vector.tensor_copy(out_sb[:, bass.ts(n, CH)],
                                  out_ps[:, bass.ts(n, CH)])
        nc.sync.dma_start(out_d.ap(), out_sb[:])

    nc.compile()
    return nc


def _get_nc():
    if "nc" not in _CACHE:
        _CACHE["nc"] = _build_nc()
    return _CACHE["nc"]


def kernel(x, Wk, Wq, Wv):
    x = np.ascontiguousarray(x, dtype=np.float32)
    assert x.shape == (B, T, E)
    nc = _get_nc()

    wqk = _to_fp32r(np.concatenate([Wq.T, Wk.T], axis=1))        # [E, 128]
    wv = _to_fp32r(Wv.T)                                          # [E, 64]
    mask = np.triu(np.ones((128, 128), dtype=np.float32))         # keep t_k <= t_q
    ident = np.eye(64, dtype=np.float32)
    ones = np.ones((128, 1), dtype=np.float32)

    in_maps = []
    for b in range(B):
        in_maps.append({
            "xt": _to_fp32r(x[b].T),
            "wqk": wqk,
            "wv": wv,
            "mask": mask,
            "ident": ident,
            "ones": ones,
        })

    res = run_bass_kernel_spmd(nc, in_maps, list(range(B)))
    out = np.empty((B, T, H), dtype=np.float32)
    for b in range(B):
        y = res.results[b]["out"]          # [65, T] unnormalized
        out[b] = (y[:H] / y[H:H + 1]).T
    return out


def run_traced(x, Wk, Wq, Wv):
    """Like kernel() but with NTFF profiling; returns (out, BassKernelResults)."""
    import types
    import antenv
    if "antenv.axon_hooks" not in sys.modules:
        hooks_mod = types.ModuleType("antenv.axon_hooks")
        _HOOK = [None]
        hooks_mod.set_axon_ntff_profile_hook = lambda h: _HOOK.__setitem__(0, h)
        hooks_mod.get_axon_ntff_profile_hook = lambda: _HOOK[0]
        sys.modules["antenv.axon_hooks"] = hooks_mod
        antenv.axon_hooks = hooks_mod
        from trn_agent_boot.trn_boot import _ntff_profile_via_ctypes
        hooks_mod.set_axon_ntff_profile_hook(
            _ntff_profile_via_ctypes("/opt/axon/libaxon_pjrt.so"))

    x = np.ascontiguousarray(x, dtype=np.float32)
    nc = _get_nc()
    wqk = _to_fp32r(np.concatenate([Wq.T, Wk.T], axis=1))
    wv = _to_fp32r(Wv.T)
    mask = np.triu(np.ones((128, 128), dtype=np.float32))
    ident = np.eye(64, dtype=np.float32)
    ones = np.ones((128, 1), dtype=np.float32)
    in_maps = [{
        "xt": _to_fp32r(x[b].T),
        "wqk": wqk, "wv": wv, "mask": mask, "ident": ident, "ones": ones,
    } for b in range(B)]
    res = run_bass_kernel_spmd(
        nc, in_maps, list(range(B)), trace=True, trace_cores=[0])
    out = np.empty((B, T, H), dtype=np.float32)
    for b in range(B):
        y = res.results[b]["out"]
        out[b] = (y[:H] / y[H:H + 1]).T
    return out, res


# revision 10
# speedup vs baseline: 1.5769x; 1.0511x over previous
"""Single-head causal attention kernel for Trainium2, 8-core data parallel.

Problem: x [8, 2048, 1024], Wk/Wq/Wv [64, 1024] ->
  out[b] = softmax(causal((x[b] @ Wq.T) @ (x[b] @ Wk.T).T / 8)) @ (x[b] @ Wv.T)

Sharding: one batch element per NeuronCore (data parallel across batch).

Per-core dataflow (all SBUF-resident, fp32):
  - host supplies xT = x[b].T [1024, 2048] so the embedding (contraction) dim
    lands on SBUF partitions directly; weights supplied pre-transposed and
    q/k fused: wqk = [Wq.T | Wk.T] [1024, 128].
  - qT/kT [64, 2048] computed with ONE packed matmul chain (stationary
    [128e, 128(q|k)] at full PE width); vT [64, 2048] separately.
  - v is re-transposed to natural [t_k, 64] via PE transpose, with a column
    of ones appended -> ve [t_k, 65]; the ones column makes the attention
    output matmul produce softmax row-sums for free.
  - scores are computed TRANSPOSED, sT[t_k, t_q] = k_j @ qT, so that
    P = exp(sT/8) needs no per-column bias (scores are bounded ~[-3, 4],
    max-subtraction is unnecessary in fp32) and P feeds the output matmul
    as the moving operand with no further transposes:
      out_psum[65, t_q] += ve_j.T @ P_j   (accumulated over key blocks j)
  - causal structure at 128-block granularity: only blocks t_k <= t_q are
    computed (136 of 256); the diagonal block is masked with a 0/1
    upper-triangular mask after exp.
  - device output is the unnormalized [65, 2048] (64 head dims + sums row);
    host divides by the sums row and transposes (0.26% of the FLOPs).
"""
import sys

for _p in ("/opt/trn_rl_repo",):
    if _p not in sys.path:
        sys.path.insert(0, _p)

import numpy as np
from contextlib import ExitStack

import concourse.bass as bass
import concourse.tile as tile
from concourse import bacc, mybir
from concourse.bass_utils import run_bass_kernel_spmd

FP = mybir.dt.float32
FPR = mybir.dt.float32r
B, T, E, H = 8, 2048, 1024, 64
NE = E // 128          # 8 e-tiles (contraction)
NT = T // 128          # 16 token tiles
CH = 512               # qkv column chunk (= one PSUM bank of fp32)
NC_CHUNKS = T // CH    # 4
SCALE = 1.0 / np.sqrt(H)  # 0.125

_CACHE = {}


def _to_fp32r(a):
    """Round fp32 to the fp32r grid (11 mantissa bits, round-to-nearest)."""
    u = np.ascontiguousarray(a, dtype=np.float32).view(np.uint32)
    u = (u + (((u >> 12) & 1) + 0x7FF)) & np.uint32(0xFFFFF000)
    return u.view(np.float32)


def _build_nc():
    nc = bacc.Bacc(None, target_bir_lowering=False, debug=False)

    xt_d = nc.dram_tensor("xt", [E, T], FPR, kind="ExternalInput")
    wqk_d = nc.dram_tensor("wqk", [E, 2 * H], FPR, kind="ExternalInput")
    wv_d = nc.dram_tensor("wv", [E, H], FPR, kind="ExternalInput")
    mask_d = nc.dram_tensor("mask", [128, 128], FPR, kind="ExternalInput")
    id_d = nc.dram_tensor("ident", [64, 64], FPR, kind="ExternalInput")
    ones_d = nc.dram_tensor("ones", [128, 1], FPR, kind="ExternalInput")
    out_d = nc.dram_tensor("out", [H + 1, T], FP, kind="ExternalOutput")

    with tile.TileContext(nc) as tc, ExitStack() as ctx:
        const = ctx.enter_context(tc.tile_pool(name="const", bufs=1))
        ve_pool = ctx.enter_context(tc.tile_pool(name="ve", bufs=NT))
        p_pool = ctx.enter_context(tc.tile_pool(name="pstripe", bufs=2))
        qk_psum = ctx.enter_context(
            tc.tile_pool(name="qk_ps", bufs=1, space=bass.MemorySpace.PSUM))
        vt_psum = ctx.enter_context(
            tc.tile_pool(name="vt_ps", bufs=1, space=bass.MemorySpace.PSUM))
        work_psum = ctx.enter_context(
            tc.tile_pool(name="work_ps", bufs=2, space=bass.MemorySpace.PSUM))
        out_psum = ctx.enter_context(
            tc.tile_pool(name="out_ps", bufs=1, space=bass.MemorySpace.PSUM))

        # ---- SBUF tensors ----
        # per-chunk tiles: Tile dependency tracking is tile-granular, so a
        # single big tile written by 4 chunk DMAs would stall every reader
        # until ALL chunks landed; separate tiles let compute start per chunk
        xts = [const.tile([128, NE * CH], FPR, name=f"xts{n}")
               for n in range(NC_CHUNKS)]
        wqk_sb = const.tile([128, NE * 2 * H], FPR)
        wv_sb = const.tile([128, NE * H], FPR)
        mask_sb = const.tile([128, 128], FPR)
        id_sb = const.tile([64, 64], FPR)
        qks = [const.tile([128, CH], FPR, name=f"qks{n}")
               for n in range(NC_CHUNKS)]             # rows 0:64 qT, 64:128 kT
        k_los = [const.tile([64, CH], FPR, name=f"klo{n}")
                 for n in range(NC_CHUNKS)]           # kT at partitions 0:64
        vTs = [const.tile([64, CH], FPR, name=f"vts{n}")
               for n in range(NC_CHUNKS)]
        out_sb = const.tile([H + 1, T], FP)
        ve = [ve_pool.tile([128, H + 1], FPR, tag="ve", name=f"ve{t}")
              for t in range(NT)]

        # ---- input DMAs ----
        nc.sync.dma_start(
            wqk_sb[:].rearrange("p (ne m) -> p ne m", m=2 * H),
            wqk_d.ap().rearrange("(ne p) m -> p ne m", p=128))
        nc.sync.dma_start(
            wv_sb[:].rearrange("p (ne m) -> p ne m", m=H),
            wv_d.ap().rearrange("(ne p) m -> p ne m", p=128))
        nc.sync.dma_start(mask_sb[:], mask_d.ap())
        nc.sync.dma_start(id_sb[:], id_d.ap())
        # x chunks, last column chunk first (attention unlocks high-j first)
        xt_in = xt_d.ap().rearrange("(ne p) t -> p ne t", p=128)
        prev_dma = None
        for n in range(NC_CHUNKS - 1, -1, -1):
            d = nc.sync.dma_start(
                xts[n][:].rearrange("p (ne t) -> p ne t", t=CH),
                xt_in[:, :, n * CH:(n + 1) * CH])
            if prev_dma is not None:
                # serialize the x chunk loads: the SDMA engines otherwise
                # round-robin all queued DMAs and no chunk finishes early
                tile.add_dep_helper(d.ins, prev_dma.ins, sync=True,
                                    reason="x chunks complete in order")
            prev_dma = d

        for t in range(NT):
            nc.scalar.dma_start(ve[t][:, H:H + 1], ones_d.ap())

        out_ps = out_psum.tile([H + 1, T], FP)

        # 512-aligned piece list for the [j*128, T) column stripe of block j:
        # a leading 128-wide diagonal piece, then pieces up to the next
        # 512 boundary, then full 512s (PSUM-bank-aligned for out_ps).
        def stripe_pieces(j):
            pieces = [(j * 128, (j + 1) * 128)]
            c = (j + 1) * 128
            while c < T:
                e = min((c // CH + 1) * CH, T)
                pieces.append((c, e))
                c = e
            return pieces

        for n in range(NC_CHUNKS - 1, -1, -1):
            # -- packed q|k projection for this column chunk --
            qk_ps = qk_psum.tile([128, CH], FP, tag="qk", name="qk_ps")
            for e in range(NE):
                nc.tensor.matmul(
                    qk_ps[:],
                    wqk_sb[:, bass.ts(e, 2 * H)],
                    xts[n][:, bass.ts(e, CH)],
                    start=(e == 0), stop=(e == NE - 1))
            nc.scalar.copy(qks[n][:], qk_ps[:])
            # move kT rows down to partitions 0:64 (partition remap via DMA)
            nc.scalar.dma_start(k_los[n][:], qks[n][64:128, :])
            # -- vT projection --
            vt_ps = vt_psum.tile([64, CH], FP, tag="vt", name="vt_ps")
            for e in range(NE):
                nc.tensor.matmul(
                    vt_ps[:],
                    wv_sb[:, bass.ts(e, H)],
                    xts[n][:, bass.ts(e, CH)],
                    start=(e == 0), stop=(e == NE - 1))
            nc.scalar.copy(vTs[n][:], vt_ps[:])
            # -- v natural tiles (PE transpose) + ones column --
            for t in range(4 * n, 4 * n + 4):
                tr_ps = work_psum.tile([128, CH], FPR, tag="work", name="tr_ps")
                nc.tensor.transpose(
                    tr_ps[:, 0:H], vTs[n][:, bass.ts(t - 4 * n, 128)], id_sb[:])
                nc.vector.tensor_copy(ve[t][:, 0:H], tr_ps[:, 0:H])

            # -- attention for key blocks j in this chunk (descending) --
            for j in range(4 * n + 3, 4 * n - 1, -1):
                pieces = stripe_pieces(j)
                stripe = p_pool.tile([128, T], FPR, tag="p")
                for (c0, c1) in pieces:
                    m = c0 // CH
                    s_ps = work_psum.tile([128, CH], FP, tag="work", name="s_ps")
                    nc.tensor.matmul(
                        s_ps[:, 0:c1 - c0],
                        k_los[j // 4][:, bass.ts(j % 4, 128)],
                        qks[m][0:64, c0 - m * CH: c1 - m * CH],
                        start=True, stop=True)
                    nc.scalar.activation(
                        stripe[:, c0 - j * 128: c1 - j * 128],
                        s_ps[:, 0:c1 - c0],
                        mybir.ActivationFunctionType.Exp,
                        scale=float(SCALE))
                # mask the diagonal block (upper-tri keep in [t_k, t_q])
                nc.vector.tensor_mul(
                    stripe[:, 0:128], stripe[:, 0:128], mask_sb[:])
                # out_psum[:, c0:c1] += ve_j.T @ P_j
                # start=True zeroes the target PSUM *bank*, so only the first
                # stripe to touch a bank (j % 4 == 3, descending) may set it;
                # stop marks the last write per bank (all j==0 pieces except
                # the diagonal one, whose bank is finished by the next piece).
                for (c0, c1) in pieces:
                    nc.tensor.matmul(
                        out_ps[:, c0:c1],
                        ve[j][:],
                        stripe[:, c0 - j * 128: c1 - j * 128],
                        start=(c0 == j * 128 and j % 4 == 3),
                        stop=(j == 0 and c0 != 0),
                        skip_group_check=True)

        for n in range(NC_CHUNKS):
            nc.vector.tensor_copy(out_sb[:, bass.ts(n, CH)],
                                  out_ps[:, bass.ts(n, CH)])
        nc.sync.dma_start(out_d.ap(), out_sb[:])

    nc.compile()
    return nc


def _get_nc():
    if "nc" not in _CACHE:
        _CACHE["nc"] = _build_nc()
    return _CACHE["nc"]


def kernel(x, Wk, Wq, Wv):
    x = np.ascontiguousarray(x, dtype=np.float32)
    assert x.shape == (B, T, E)
    nc = _get_nc()

    wqk = _to_fp32r(np.concatenate([Wq.T, Wk.T], axis=1))        # [E, 128]
    wv = _to_fp32r(Wv.T)                                          # [E, 64]
    mask = np.triu(np.ones((128, 128), dtype=np.float32))         # keep t_k <= t_q
    ident = np.eye(64, dtype=np.float32)
    ones = np.ones((128, 1), dtype=np.float32)

    in_maps = []
    for b in range(B):
        in_maps.append({
            "xt": _to_fp32r(x[b].T),
            "wqk": wqk,
            "wv": wv,
            "mask": mask,
            "ident": ident,
            "ones": ones,
        })

    res = run_bass_kernel_spmd(nc, in_maps, list(range(B)))
    out = np.empty((B, T, H), dtype=np.float32)
    for b in range(B):
        y = res.results[b]["out"]          # [65, T] unnormalized
        out[b] = (y[:H] / y[H:H + 1]).T
    return out


def run_traced(x, Wk, Wq, Wv):
    """Like kernel() but with NTFF profiling; returns (out, BassKernelResults)."""
    import types
    import antenv
    if "antenv.axon_hooks" not in sys.modules:
        hooks_mod = types.ModuleType("antenv.axon_hooks")
        _HOOK = [None]
        hooks_mod.set_axon_ntff_profile_hook = lambda h: _HOOK.__setitem__(0, h)
        hooks_mod.get_axon_ntff_profile_hook = lambda: _HOOK[0]
        sys.modules["antenv.axon_hooks"] = hooks_mod
        antenv.axon_hooks = hooks_mod
        from trn_agent_boot.trn_boot import _ntff_profile_via_ctypes
        hooks_mod.set_axon_ntff_profile_hook(
            _ntff_profile_via_ctypes("/opt/axon/libaxon_pjrt.so"))

    x = np.ascontiguousarray(x, dtype=np.float32)
    nc = _get_nc()
    wqk = _to_fp32r(np.concatenate([Wq.T, Wk.T], axis=1))
    wv = _to_fp32r(Wv.T)
    mask = np.triu(np.ones((128, 128), dtype=np.float32))
    ident = np.eye(64, dtype=np.float32)
    ones = np.ones((128, 1), dtype=np.float32)
    in_maps = [{
        "xt": _to_fp32r(x[b].T),
        "wqk": wqk, "wv": wv, "mask": mask, "ident": ident, "ones": ones,
    } for b in range(B)]
    res = run_bass_kernel_spmd(
        nc, in_maps, list(range(B)), trace=True, trace_cores=[0])
    out = np.empty((B, T, H), dtype=np.float32)
    for b in range(B):
        y = res.results[b]["out"]
        out[b] = (y[:H] / y[H:H + 1]).T
    return out, res
